# revision 13
# baseline (speedup 1.0000x reference)
"""Chamfer completion-loss kernel for Trainium2 (8 NeuronCores).

Math: for pred set A and target set B,
  chamfer(A, B) = mean_a min_b ||a-b|| + mean_b min_a ||a-b||
  loss = mean_batch( chamfer(fine, target) + 0.5 * chamfer(coarse, target) )

Device strategy:
  - Work in NEGATED squared-distance space S = 2 a.b - |a|^2 - |b|^2 = -d^2,
    computed by matmul with augmented vectors
      stationary u = [a, |a|^2, 1],  moving v = [2b, -1, -|b|^2]
    so min_d^2 = -max_S, and only free-dim MAX-reduces are needed.
  - fp32 matmuls cost 4 PE cycles/row; bf16 costs 1.  To get fp32-grade
    accuracy at bf16 speed, split each augmented vector into a bf16 high
    and bf16 residual part (u = u0 + u1, v = v0 + v1) and stack the three
    significant cross terms along the contraction dim:
      K=30:  u' = [u0,u0,u1,u1,u0,u2],  v' = [v0,v1,v0,v1,v2,v0]
      (all cross terms through second order; error ~2^-24|u||v|)
    Matmul cost depends only on moving rows, not K, so this is 4x faster
    than fp32 at ~5e-4 absolute error on S (loss rel err ~1e-5).
  - Reductions: hardware allows only ONE PSUM operand per DVE op, so the
    max-reduce of PSUM is spread over every engine that can touch it.
    Each stat tile owns 4 psum tiles [128, 2048]; a per-stat-tile "type"
    decides the consumer mix:
      '4': ACT copies tiles 0,2 to SBUF bf16; DVE ttr pairs tile 1,3
           halves with the copies (max+max fused, accum to partial col)
      'd': like '4' but the copies go over DMA (psum -> sbuf f32)
      '6': ACT copies all 4; GPSIMD folds 8192 -> 128 with tensor_tensor
           max; DVE does one tiny reduce
      '5': ACT copies 0,1,2; GP pre-folds two copies, DVE pairs tile 3
           with them; GP folds the third copy down alone
    The pattern string mixes types so DVE/ACT/GP/DMA all stay under the
    PE's matmul time and the kernel stays PE-bound.
  - Two matmul passes per batch: preds-stationary (row mins) and
    targets-stationary (col mins); both reduce along the free dim.
  - Shard: core i owns fine rows [i*1024:(i+1)*1024], coarse rows
    [i*128:(i+1)*128], target rows [i*1024:(i+1)*1024] of every batch.
    Each core sees the full opposing set, so no cross-core combining of
    mins is needed; host just concatenates and finishes with sqrt/means.
"""
import numpy as np

ALPHA = 0.5
B = 4
NF, NC_, NT = 8192, 1024, 8192
M = 8                      # cores
FS, CS, TS = NF // M, NC_ // M, NT // M   # per-core rows: 1024, 128, 1024
K = 30                     # stacked split contraction dim
CHUNK = 512                # moving free-dim per matmul (one PSUM bank)
PST = 2048                 # psum tile width (4 banks)
PAT = "4"                  # unused knob (kept for timing harness compat)

_CACHE = {}


def _build_nc(repeat=1, pat=PAT):
    import concourse.bacc as bacc
    import concourse.tile as tile
    from concourse import mybir

    F32 = mybir.dt.float32
    BF16 = mybir.dt.bfloat16
    AX = mybir.AxisListType.X
    MAX = mybir.AluOpType.max
    COPY = mybir.ActivationFunctionType.Copy
    NEG = -3.0e38

    nc = bacc.Bacc(None, target_bir_lowering=False)

    d_fstat = nc.dram_tensor("fstat", [K, B * FS], BF16, kind="ExternalInput")
    d_cstat = nc.dram_tensor("cstat", [K, B * CS], BF16, kind="ExternalInput")
    d_tstat = nc.dram_tensor("tstat", [K, B * TS], BF16, kind="ExternalInput")
    d_tmov = nc.dram_tensor("tmov", [B, K, NT], BF16, kind="ExternalInput")
    d_fmov = nc.dram_tensor("fmov", [B, K, NF], BF16, kind="ExternalInput")
    d_cmov = nc.dram_tensor("cmov", [B, K, NC_], BF16, kind="ExternalInput")

    # outputs hold max-of-S per point, laid out [partition, tile] (host reorders)
    d_ofr = nc.dram_tensor("o_fr", [B, FS], F32, kind="ExternalOutput")
    d_ocr = nc.dram_tensor("o_cr", [B, CS], F32, kind="ExternalOutput")
    d_ocf = nc.dram_tensor("o_cf", [B, TS], F32, kind="ExternalOutput")
    d_occ = nc.dram_tensor("o_cc", [B, TS], F32, kind="ExternalOutput")

    FT = FS // 128       # 8 fine tiles per core-batch
    TT = TS // 128       # 8 target tiles per core-batch
    NG = NT // PST       # 4 psum tiles per stat tile
    CPG = PST // CHUNK   # 4 matmul chunks per psum tile
    H = PST // 2

    with tile.TileContext(nc) as tc:
        with (
            tc.tile_pool(name="stats", bufs=1) as stats,
            tc.tile_pool(name="movs", bufs=2) as movs,
            tc.tile_pool(name="parts", bufs=4) as partp,
            tc.tile_pool(name="coll", bufs=2) as coll,
            tc.tile_pool(name="gbig", bufs=4) as gbp,
            tc.tile_pool(name="gsm", bufs=4) as gsp,
            tc.tile_pool(name="bcopy", bufs=8) as bcp,
            tc.tile_pool(name="ps", bufs=2, space="PSUM") as psp,
        ):
            sb_fstat = stats.tile([K, B * FS], BF16)
            sb_cstat = stats.tile([K, B * CS], BF16)
            sb_tstat = stats.tile([K, B * TS], BF16)

            state = {"ctr": 0}

            def mm_tile(stat_ap, mov, base_ch):
                """Fill one psum tile [128, PST] with CPG matmul chunks."""
                ps = psp.tile([128, PST], F32)
                for c in range(CPG):
                    ch = base_ch + c
                    nc.tensor.matmul(
                        ps[:, c * CHUNK:(c + 1) * CHUNK],
                        stat_ap,
                        mov[:, ch * CHUNK:(ch + 1) * CHUNK],
                        start=True, stop=True,
                    )
                return ps

            def fold_chain(src, width, pcol):
                """DVE tt-fold src [128, width] bf16 down to 128, reduce to pcol."""
                cur = src
                w = width
                while w > 128:
                    nxt = gsp.tile([128, w // 2], BF16)
                    nc.vector.tensor_tensor(nxt[:], cur[:, 0:w // 2], cur[:, w // 2:w], op=MAX)
                    cur = nxt
                    w //= 2
                nc.vector.tensor_reduce(pcol, cur[:], axis=AX, op=MAX)

            def stat_group(stat_ap, mov, dst_col):
                """All NG psum tiles of one stat tile -> collector column.

                Tile 0 is plain-reduced by DVE straight from PSUM; tiles
                1,2,3 are copied to SBUF bf16 by ACT and folded by DVE
                tensor_tensor max (2x mode on packed bf16)."""
                parts = partp.tile([128, 2], F32)
                ps0 = mm_tile(stat_ap, mov, 0)
                nc.vector.tensor_reduce(parts[:, 0:1], ps0[:], axis=AX, op=MAX)
                bcs = []
                for g in range(1, NG):
                    ps = mm_tile(stat_ap, mov, g * CPG)
                    bc = bcp.tile([128, PST], BF16)
                    nc.scalar.activation(bc[:], ps[:], COPY)
                    bcs.append(bc)
                m1 = gbp.tile([128, PST], BF16)
                nc.vector.tensor_tensor(m1[:], bcs[0][:], bcs[1][:], op=MAX)
                m2 = gbp.tile([128, PST], BF16)
                nc.vector.tensor_tensor(m2[:], m1[:], bcs[2][:], op=MAX)
                fold_chain(m2, PST, parts[:, 1:2])
                nc.vector.tensor_reduce(dst_col, parts[:], axis=AX, op=MAX)

            def whole_body():
              nc.sync.dma_start(sb_fstat[:], d_fstat[:])
              nc.sync.dma_start(sb_cstat[:], d_cstat[:])
              nc.sync.dma_start(sb_tstat[:], d_tstat[:])
              for b in range(B):
                sb_tmov = movs.tile([K, NT], BF16)
                sb_fmov = movs.tile([K, NF], BF16)
                sb_cmov = movs.tile([K, NC_], BF16)
                nc.sync.dma_start(sb_tmov[:], d_tmov[b])
                nc.sync.dma_start(sb_fmov[:], d_fmov[b])
                nc.sync.dma_start(sb_cmov[:], d_cmov[b])

                cfr = coll.tile([128, FT], F32)
                ccr = coll.tile([128, 1], F32)
                ccf = coll.tile([128, TT], F32)
                ccc = coll.tile([128, TT], F32)

                # ---- pass R: preds stationary, targets moving -> row maxes
                for t in range(FT + 1):
                    if t < FT:
                        stat_ap = sb_fstat[:, (b * FT + t) * 128:(b * FT + t + 1) * 128]
                        dst = cfr[:, t:t + 1]
                    else:
                        stat_ap = sb_cstat[:, b * CS:(b + 1) * CS]
                        dst = ccr[:, 0:1]
                    stat_group(stat_ap, sb_tmov, dst)

                # ---- pass C: targets stationary; fine then coarse moving
                for t in range(TT):
                    stat_ap = sb_tstat[:, (b * TT + t) * 128:(b * TT + t + 1) * 128]
                    stat_group(stat_ap, sb_fmov, ccf[:, t:t + 1])

                    # coarse moving: 2 chunks in one psum tile; alternate the
                    # consumer between DVE-direct and ACT+GP to balance load
                    ps = psp.tile([128, PST], F32)
                    for c in range(NC_ // CHUNK):
                        nc.tensor.matmul(
                            ps[:, c * CHUNK:(c + 1) * CHUNK],
                            stat_ap,
                            sb_cmov[:, c * CHUNK:(c + 1) * CHUNK],
                            start=True, stop=True,
                        )
                    if t % 2 == 0:
                        nc.vector.tensor_reduce(
                            ccc[:, t:t + 1], ps[:, 0:NC_], axis=AX, op=MAX)
                    else:
                        bc = bcp.tile([128, NC_], BF16)
                        nc.scalar.activation(bc[:], ps[:, 0:NC_], COPY)
                        fold_chain(bc, NC_, ccc[:, t:t + 1])

                nc.sync.dma_start(d_ofr[b], cfr[:])
                nc.sync.dma_start(d_ocr[b], ccr[:])
                nc.sync.dma_start(d_ocf[b], ccf[:])
                nc.sync.dma_start(d_occ[b], ccc[:])

            if repeat == 1:
                whole_body()
            else:
                with tc.For_i(0, repeat, 1):
                    whole_body()
    nc.finalize()
    return nc


def _bf16_split3(x):
    """f32 [..] -> (hi, mid, lo residuals) all bf16 via ml_dtypes."""
    import ml_dtypes
    BF = ml_dtypes.bfloat16
    x = x.astype(np.float32)
    hi = x.astype(BF)
    r = x - hi.astype(np.float32)
    mid = r.astype(BF)
    lo = (r - mid.astype(np.float32)).astype(BF)
    return hi, mid, lo


def _stat_aug_split(x):
    # [B, N, 3] -> [B, K, N] bf16, u' = [u0, u0, u1] for u = [x, y, z, |p|^2, 1]
    b, n, _ = x.shape
    u = np.empty((b, 5, n), np.float32)
    u[:, 0:3] = np.transpose(x, (0, 2, 1))
    u[:, 3] = np.sum(x.astype(np.float64) ** 2, axis=-1)
    u[:, 4] = 1.0
    u0, u1, u2 = _bf16_split3(u)
    return np.concatenate([u0, u0, u1, u1, u0, u2], axis=1)


def _mov_aug_split(x):
    # [B, N, 3] -> [B, K, N] bf16, v' = [v0, v1, v0] for v = [2x,2y,2z, -1, -|p|^2]
    b, n, _ = x.shape
    v = np.empty((b, 5, n), np.float32)
    v[:, 0:3] = 2.0 * np.transpose(x, (0, 2, 1))
    v[:, 3] = -1.0
    v[:, 4] = -np.sum(x.astype(np.float64) ** 2, axis=-1)
    v0, v1, v2 = _bf16_split3(v)
    return np.concatenate([v0, v1, v0, v1, v2, v0], axis=1)


def _detile(a):
    # device layout [B, 128*T] indexed p*T + t  ->  local row order t*128 + p
    b, n = a.shape
    t = n // 128
    return a.reshape(b, 128, t).transpose(0, 2, 1).reshape(b, n)


def _get_runner():
    if "nc" not in _CACHE:
        _CACHE["nc"] = _build_nc()
    return _CACHE["nc"]


def make_in_maps(fine, coarse, target):
    """Host pre-processing: split-augment, per-core stationary slices."""
    fstat = _stat_aug_split(fine)     # [B, K, NF]
    cstat = _stat_aug_split(coarse)
    tstat = _stat_aug_split(target)
    tmov = _mov_aug_split(target)
    fmov = _mov_aug_split(fine)
    cmov = _mov_aug_split(coarse)

    def stat_slice(s, i, n):
        # [B, K, N] -> core slice [K, B*n] (batch-major columns)
        sl = s[:, :, i * n:(i + 1) * n]            # [B, K, n]
        return np.ascontiguousarray(sl.transpose(1, 0, 2).reshape(K, B * n))

    in_maps = []
    for i in range(M):
        in_maps.append({
            "fstat": stat_slice(fstat, i, FS),
            "cstat": stat_slice(cstat, i, CS),
            "tstat": stat_slice(tstat, i, TS),
            "tmov": tmov,
            "fmov": fmov,
            "cmov": cmov,
        })
    return in_maps


def run_device(fine, coarse, target):
    """Run the device part; returns per-core raw outputs (list of dicts)."""
    from concourse.bass_utils import run_bass_kernel_spmd

    nc = _get_runner()
    in_maps = make_in_maps(fine, coarse, target)
    res = run_bass_kernel_spmd(nc, in_maps, core_ids=list(range(M)))
    return res.results


def finish(results):
    """Combine per-core S-max outputs into the scalar loss."""
    fr = np.concatenate([_detile(r["o_fr"]) for r in results], axis=1)  # [B, NF]
    cr = np.concatenate([r["o_cr"] for r in results], axis=1)           # [B, NC]
    cf = np.concatenate([_detile(r["o_cf"]) for r in results], axis=1)  # [B, NT]
    cc = np.concatenate([_detile(r["o_cc"]) for r in results], axis=1)  # [B, NT]

    def dmin(s):
        return np.sqrt(np.maximum(-s.astype(np.float64), 0.0))

    fine_loss = dmin(fr).mean(axis=1) + dmin(cf).mean(axis=1)
    coarse_loss = dmin(cr).mean(axis=1) + dmin(cc).mean(axis=1)
    loss = (fine_loss + ALPHA * coarse_loss).mean()
    return np.float32(loss)


def kernel(fine, coarse, target):
    fine = np.asarray(fine, np.float32)
    coarse = np.asarray(coarse, np.float32)
    target = np.asarray(target, np.float32)
    return finish(run_device(fine, coarse, target))


# revision 18
# speedup vs baseline: 1.5396x; 1.5396x over previous
"""Chamfer completion-loss kernel for Trainium2 (8 NeuronCores).

Math: for pred set A and target set B,
  chamfer(A, B) = mean_a min_b ||a-b|| + mean_b min_a ||a-b||
  loss = mean_batch( chamfer(fine, target) + 0.5 * chamfer(coarse, target) )

Device strategy:
  - Work in NEGATED squared-distance space S = 2 a.b - |a|^2 - |b|^2 = -d^2,
    computed by matmul with augmented vectors
      stationary u = [a, |a|^2, 1],  moving v = [2b, -1, -|b|^2]
    so min_d^2 = -max_S, and only free-dim MAX-reduces are needed.
  - fp32 matmuls cost 4 PE cycles/row; bf16 costs 1.  To get fp32-grade
    accuracy at bf16 speed, split each augmented vector into a bf16 high
    and bf16 residual part (u = u0 + u1, v = v0 + v1) and stack the three
    significant cross terms along the contraction dim:
      K=30:  u' = [u0,u0,u1,u1,u0,u2],  v' = [v0,v1,v0,v1,v2,v0]
      (all cross terms through second order; error ~2^-24|u||v|)
    Matmul cost depends only on moving rows, not K, so this is 4x faster
    than fp32 at ~5e-4 absolute error on S (loss rel err ~1e-5).
  - Reductions: hardware allows only ONE PSUM operand per DVE op, so the
    max-reduce of PSUM is spread over every engine that can touch it.
    Each stat tile owns 4 psum tiles [128, 2048]; a per-stat-tile "type"
    decides the consumer mix:
      '4': ACT copies tiles 0,2 to SBUF bf16; DVE ttr pairs tile 1,3
           halves with the copies (max+max fused, accum to partial col)
      'd': like '4' but the copies go over DMA (psum -> sbuf f32)
      '6': ACT copies all 4; GPSIMD folds 8192 -> 128 with tensor_tensor
           max; DVE does one tiny reduce
      '5': ACT copies 0,1,2; GP pre-folds two copies, DVE pairs tile 3
           with them; GP folds the third copy down alone
    The pattern string mixes types so DVE/ACT/GP/DMA all stay under the
    PE's matmul time and the kernel stays PE-bound.
  - Two matmul passes per batch: preds-stationary (row mins) and
    targets-stationary (col mins); both reduce along the free dim.
  - Shard: core i owns fine rows [i*1024:(i+1)*1024], coarse rows
    [i*128:(i+1)*128], target rows [i*1024:(i+1)*1024] of every batch.
    Each core sees the full opposing set, so no cross-core combining of
    mins is needed; host just concatenates and finishes with sqrt/means.
"""
import numpy as np

ALPHA = 0.5
B = 4
NF, NC_, NT = 8192, 1024, 8192
M = 8                      # cores
FS, CS, TS = NF // M, NC_ // M, NT // M   # per-core rows: 1024, 128, 1024
K = 30                     # stacked split contraction dim
CHUNK = 512                # moving free-dim per matmul (one PSUM bank)
PST = 2048                 # psum tile width (4 banks)
PAT = "4"                  # unused knob (kept for timing harness compat)

_CACHE = {}


def _build_nc(repeat=1, pat=PAT, mode='full', ps_bufs=4, ps_width=1024):
    import concourse.bacc as bacc
    import concourse.tile as tile
    from concourse import mybir

    F32 = mybir.dt.float32
    BF16 = mybir.dt.bfloat16
    AX = mybir.AxisListType.X
    MAX = mybir.AluOpType.max
    COPY = mybir.ActivationFunctionType.Copy
    NEG = -3.0e38

    nc = bacc.Bacc(None, target_bir_lowering=False)

    d_fstat = nc.dram_tensor("fstat", [K, B * FS], BF16, kind="ExternalInput")
    d_cstat = nc.dram_tensor("cstat", [K, B * CS], BF16, kind="ExternalInput")
    d_tstat = nc.dram_tensor("tstat", [K, B * TS], BF16, kind="ExternalInput")
    d_tmov = nc.dram_tensor("tmov", [B, K, NT], BF16, kind="ExternalInput")
    d_fmov = nc.dram_tensor("fmov", [B, K, NF], BF16, kind="ExternalInput")
    d_cmov = nc.dram_tensor("cmov", [B, K, NC_], BF16, kind="ExternalInput")

    # outputs hold max-of-S per point, laid out [partition, tile] (host reorders)
    d_ofr = nc.dram_tensor("o_fr", [B, FS], F32, kind="ExternalOutput")
    d_ocr = nc.dram_tensor("o_cr", [B, CS], F32, kind="ExternalOutput")
    d_ocf = nc.dram_tensor("o_cf", [B, TS], F32, kind="ExternalOutput")
    d_occ = nc.dram_tensor("o_cc", [B, TS], F32, kind="ExternalOutput")

    FT = FS // 128       # 8 fine tiles per core-batch
    TT = TS // 128       # 8 target tiles per core-batch
    NG = NT // PST       # 4 psum tiles per stat tile
    CPG = PST // CHUNK   # 4 matmul chunks per psum tile
    H = PST // 2

    with tile.TileContext(nc) as tc:
        with (
            tc.tile_pool(name="stats", bufs=1) as stats,
            tc.tile_pool(name="movs", bufs=2) as movs,
            tc.tile_pool(name="parts", bufs=4) as partp,
            tc.tile_pool(name="coll", bufs=2) as coll,
            tc.tile_pool(name="gbig", bufs=4) as gbp,
            tc.tile_pool(name="gsm", bufs=4) as gsp,
            tc.tile_pool(name="bcopy", bufs=8) as bcp,
            tc.tile_pool(name="ps", bufs=ps_bufs, space="PSUM") as psp,
        ):
            sb_fstat = stats.tile([K, B * FS], BF16)
            sb_cstat = stats.tile([K, B * CS], BF16)
            sb_tstat = stats.tile([K, B * TS], BF16)

            state = {"ctr": 0}

            def mm_tile(stat_ap, mov, base_ch):
                """Fill one psum tile [128, PST] with CPG matmul chunks."""
                ps = psp.tile([128, PST], F32)
                for c in range(CPG):
                    ch = base_ch + c
                    nc.tensor.matmul(
                        ps[:, c * CHUNK:(c + 1) * CHUNK],
                        stat_ap,
                        mov[:, ch * CHUNK:(ch + 1) * CHUNK],
                        start=True, stop=True,
                    )
                return ps

            def fold_chain(src, width, pcol):
                """DVE tt-fold src [128, width] bf16 down to 128, reduce to pcol."""
                cur = src
                w = width
                while w > 128:
                    nxt = gsp.tile([128, w // 2], BF16)
                    nc.vector.tensor_tensor(nxt[:], cur[:, 0:w // 2], cur[:, w // 2:w], op=MAX)
                    cur = nxt
                    w //= 2
                nc.vector.tensor_reduce(pcol, cur[:], axis=AX, op=MAX)

            def stat_group(stat_ap, mov, dst_col):
                """All NG psum tiles of one stat tile -> collector column.

                Tile 0 is plain-reduced by DVE straight from PSUM; tiles
                1,2,3 are copied to SBUF bf16 by ACT and folded by DVE
                tensor_tensor max (2x mode on packed bf16)."""
                if mode == 'pe':
                    # probe: minimal consumer, PE/DMA-paced, ps_width-sized slots
                    nch = PST // ps_width
                    parts = partp.tile([128, NG * nch], F32)
                    for g in range(NG * nch):
                        ps = psp.tile([128, ps_width], F32)
                        for c in range(ps_width // CHUNK):
                            ch = g * (ps_width // CHUNK) + c
                            nc.tensor.matmul(
                                ps[:, c * CHUNK:(c + 1) * CHUNK], stat_ap,
                                mov[:, ch * CHUNK:(ch + 1) * CHUNK],
                                start=True, stop=True)
                        nc.vector.tensor_reduce(parts[:, g:g + 1], ps[:, 0:64], axis=AX, op=MAX)
                    nc.vector.tensor_reduce(dst_col, parts[:], axis=AX, op=MAX)
                    return
                if mode == 'dve':
                    # probe: all tiles direct-reduced on DVE
                    parts = partp.tile([128, NG], F32)
                    for g in range(NG):
                        ps = mm_tile(stat_ap, mov, g * CPG)
                        nc.vector.tensor_reduce(parts[:, g:g + 1], ps[:], axis=AX, op=MAX)
                    nc.vector.tensor_reduce(dst_col, parts[:], axis=AX, op=MAX)
                    return
                if mode == 'act':
                    # probe: all tiles ACT-copied, tiny DVE reduces keep data live
                    parts = partp.tile([128, NG], F32)
                    for g in range(NG):
                        ps = mm_tile(stat_ap, mov, g * CPG)
                        bc = bcp.tile([128, PST], BF16)
                        nc.scalar.activation(bc[:], ps[:], COPY)
                        nc.vector.tensor_reduce(parts[:, g:g + 1], bc[:, 0:64], axis=AX, op=MAX)
                    nc.vector.tensor_reduce(dst_col, parts[:], axis=AX, op=MAX)
                    return
                if ps_width == PST:
                    parts = partp.tile([128, 2], F32)
                    ps0 = mm_tile(stat_ap, mov, 0)
                    nc.vector.tensor_reduce(parts[:, 0:1], ps0[:], axis=AX, op=MAX)
                    bcs = []
                    for g in range(1, NG):
                        ps = mm_tile(stat_ap, mov, g * CPG)
                        bc = bcp.tile([128, PST], BF16)
                        nc.scalar.activation(bc[:], ps[:], COPY)
                        bcs.append(bc)
                    m1 = gbp.tile([128, PST], BF16)
                    nc.vector.tensor_tensor(m1[:], bcs[0][:], bcs[1][:], op=MAX)
                    m2 = gbp.tile([128, PST], BF16)
                    nc.vector.tensor_tensor(m2[:], m1[:], bcs[2][:], op=MAX)
                    fold_chain(m2, PST, parts[:, 1:2])
                    nc.vector.tensor_reduce(dst_col, parts[:], axis=AX, op=MAX)
                    return
                # half-width slots: 8 tiles of [128, 1024] per stat tile;
                # tiles 0,1 DVE-direct, tiles 2..7 ACT-copied + DVE folds
                HW_ = ps_width
                nhp = HW_ // CHUNK
                parts = partp.tile([128, 3], F32)
                bcs = []
                for g in range(NT // HW_):
                    ps = psp.tile([128, HW_], F32)
                    for c in range(nhp):
                        ch = g * nhp + c
                        nc.tensor.matmul(
                            ps[:, c * CHUNK:(c + 1) * CHUNK], stat_ap,
                            mov[:, ch * CHUNK:(ch + 1) * CHUNK],
                            start=True, stop=True)
                    if g < 2:
                        nc.vector.tensor_reduce(parts[:, g:g + 1], ps[:], axis=AX, op=MAX)
                    else:
                        bc = bcp.tile([128, HW_], BF16)
                        nc.scalar.activation(bc[:], ps[:], COPY)
                        bcs.append(bc)
                h1 = gbp.tile([128, HW_], BF16)
                nc.vector.tensor_tensor(h1[:], bcs[0][:], bcs[1][:], op=MAX)
                h2 = gbp.tile([128, HW_], BF16)
                nc.vector.tensor_tensor(h2[:], bcs[2][:], bcs[3][:], op=MAX)
                h3 = gbp.tile([128, HW_], BF16)
                nc.vector.tensor_tensor(h3[:], bcs[4][:], bcs[5][:], op=MAX)
                m = gbp.tile([128, HW_], BF16)
                nc.vector.tensor_tensor(m[:], h1[:], h2[:], op=MAX)
                m2 = gbp.tile([128, HW_], BF16)
                nc.vector.tensor_tensor(m2[:], m[:], h3[:], op=MAX)
                fold_chain(m2, HW_, parts[:, 2:3])
                nc.vector.tensor_reduce(dst_col, parts[:], axis=AX, op=MAX)

            def whole_body():
              nc.sync.dma_start(sb_fstat[:], d_fstat[:])
              nc.sync.dma_start(sb_cstat[:], d_cstat[:])
              nc.sync.dma_start(sb_tstat[:], d_tstat[:])
              for b in range(B):
                sb_tmov = movs.tile([K, NT], BF16)
                sb_fmov = movs.tile([K, NF], BF16)
                sb_cmov = movs.tile([K, NC_], BF16)
                nc.sync.dma_start(sb_tmov[:], d_tmov[b])
                nc.sync.dma_start(sb_fmov[:], d_fmov[b])
                nc.sync.dma_start(sb_cmov[:], d_cmov[b])

                cfr = coll.tile([128, FT], F32)
                ccr = coll.tile([128, 1], F32)
                ccf = coll.tile([128, TT], F32)
                ccc = coll.tile([128, TT], F32)

                # ---- pass R: preds stationary, targets moving -> row maxes
                for t in range(FT + 1):
                    if t < FT:
                        stat_ap = sb_fstat[:, (b * FT + t) * 128:(b * FT + t + 1) * 128]
                        dst = cfr[:, t:t + 1]
                    else:
                        stat_ap = sb_cstat[:, b * CS:(b + 1) * CS]
                        dst = ccr[:, 0:1]
                    stat_group(stat_ap, sb_tmov, dst)

                # ---- pass C: targets stationary; fine then coarse moving
                for t in range(TT):
                    stat_ap = sb_tstat[:, (b * TT + t) * 128:(b * TT + t + 1) * 128]
                    stat_group(stat_ap, sb_fmov, ccf[:, t:t + 1])

                    # coarse moving: NC_/CHUNK chunks; alternate the consumer
                    # between DVE-direct and ACT+fold to balance load
                    if ps_width >= NC_:
                        ps = psp.tile([128, ps_width], F32)
                        for c in range(NC_ // CHUNK):
                            nc.tensor.matmul(
                                ps[:, c * CHUNK:(c + 1) * CHUNK],
                                stat_ap,
                                sb_cmov[:, c * CHUNK:(c + 1) * CHUNK],
                                start=True, stop=True,
                            )
                        if t % 2 == 0:
                            nc.vector.tensor_reduce(
                                ccc[:, t:t + 1], ps[:, 0:NC_], axis=AX, op=MAX)
                        else:
                            bc = bcp.tile([128, NC_], BF16)
                            nc.scalar.activation(bc[:], ps[:, 0:NC_], COPY)
                            fold_chain(bc, NC_, ccc[:, t:t + 1])
                    else:
                        cparts = partp.tile([128, 2], F32)
                        for c in range(NC_ // CHUNK):
                            ps = psp.tile([128, ps_width], F32)
                            nc.tensor.matmul(
                                ps[:, 0:CHUNK], stat_ap,
                                sb_cmov[:, c * CHUNK:(c + 1) * CHUNK],
                                start=True, stop=True,
                            )
                            nc.vector.tensor_reduce(
                                cparts[:, c:c + 1], ps[:, 0:CHUNK] if mode != 'pe' else ps[:, 0:64],
                                axis=AX, op=MAX)
                        nc.vector.tensor_reduce(ccc[:, t:t + 1], cparts[:], axis=AX, op=MAX)

                nc.sync.dma_start(d_ofr[b], cfr[:])
                nc.sync.dma_start(d_ocr[b], ccr[:])
                nc.sync.dma_start(d_ocf[b], ccf[:])
                nc.sync.dma_start(d_occ[b], ccc[:])

            if repeat == 1:
                whole_body()
            else:
                with tc.For_i(0, repeat, 1):
                    whole_body()
    nc.finalize()
    return nc


def _bf16_split3(x):
    """f32 [..] -> (hi, mid, lo residuals) all bf16 via ml_dtypes."""
    import ml_dtypes
    BF = ml_dtypes.bfloat16
    x = x.astype(np.float32)
    hi = x.astype(BF)
    r = x - hi.astype(np.float32)
    mid = r.astype(BF)
    lo = (r - mid.astype(np.float32)).astype(BF)
    return hi, mid, lo


def _stat_aug_split(x):
    # [B, N, 3] -> [B, K, N] bf16, u' = [u0, u0, u1] for u = [x, y, z, |p|^2, 1]
    b, n, _ = x.shape
    u = np.empty((b, 5, n), np.float32)
    u[:, 0:3] = np.transpose(x, (0, 2, 1))
    u[:, 3] = np.sum(x.astype(np.float64) ** 2, axis=-1)
    u[:, 4] = 1.0
    u0, u1, u2 = _bf16_split3(u)
    return np.concatenate([u0, u0, u1, u1, u0, u2], axis=1)


def _mov_aug_split(x):
    # [B, N, 3] -> [B, K, N] bf16, v' = [v0, v1, v0] for v = [2x,2y,2z, -1, -|p|^2]
    b, n, _ = x.shape
    v = np.empty((b, 5, n), np.float32)
    v[:, 0:3] = 2.0 * np.transpose(x, (0, 2, 1))
    v[:, 3] = -1.0
    v[:, 4] = -np.sum(x.astype(np.float64) ** 2, axis=-1)
    v0, v1, v2 = _bf16_split3(v)
    return np.concatenate([v0, v1, v0, v1, v2, v0], axis=1)


def _detile(a):
    # device layout [B, 128*T] indexed p*T + t  ->  local row order t*128 + p
    b, n = a.shape
    t = n // 128
    return a.reshape(b, 128, t).transpose(0, 2, 1).reshape(b, n)


def _get_runner():
    if "nc" not in _CACHE:
        _CACHE["nc"] = _build_nc()
    return _CACHE["nc"]


def make_in_maps(fine, coarse, target):
    """Host pre-processing: split-augment, per-core stationary slices."""
    fstat = _stat_aug_split(fine)     # [B, K, NF]
    cstat = _stat_aug_split(coarse)
    tstat = _stat_aug_split(target)
    tmov = _mov_aug_split(target)
    fmov = _mov_aug_split(fine)
    cmov = _mov_aug_split(coarse)

    def stat_slice(s, i, n):
        # [B, K, N] -> core slice [K, B*n] (batch-major columns)
        sl = s[:, :, i * n:(i + 1) * n]            # [B, K, n]
        return np.ascontiguousarray(sl.transpose(1, 0, 2).reshape(K, B * n))

    in_maps = []
    for i in range(M):
        in_maps.append({
            "fstat": stat_slice(fstat, i, FS),
            "cstat": stat_slice(cstat, i, CS),
            "tstat": stat_slice(tstat, i, TS),
            "tmov": tmov,
            "fmov": fmov,
            "cmov": cmov,
        })
    return in_maps


def run_device(fine, coarse, target):
    """Run the device part; returns per-core raw outputs (list of dicts)."""
    from concourse.bass_utils import run_bass_kernel_spmd

    nc = _get_runner()
    in_maps = make_in_maps(fine, coarse, target)
    res = run_bass_kernel_spmd(nc, in_maps, core_ids=list(range(M)))
    return res.results


def finish(results):
    """Combine per-core S-max outputs into the scalar loss."""
    fr = np.concatenate([_detile(r["o_fr"]) for r in results], axis=1)  # [B, NF]
    cr = np.concatenate([r["o_cr"] for r in results], axis=1)           # [B, NC]
    cf = np.concatenate([_detile(r["o_cf"]) for r in results], axis=1)  # [B, NT]
    cc = np.concatenate([_detile(r["o_cc"]) for r in results], axis=1)  # [B, NT]

    def dmin(s):
        return np.sqrt(np.maximum(-s.astype(np.float64), 0.0))

    fine_loss = dmin(fr).mean(axis=1) + dmin(cf).mean(axis=1)
    coarse_loss = dmin(cr).mean(axis=1) + dmin(cc).mean(axis=1)
    loss = (fine_loss + ALPHA * coarse_loss).mean()
    return np.float32(loss)


def kernel(fine, coarse, target):
    fine = np.asarray(fine, np.float32)
    coarse = np.asarray(coarse, np.float32)
    target = np.asarray(target, np.float32)
    return finish(run_device(fine, coarse, target))
